# revision 10
# baseline (speedup 1.0000x reference)
"""Trainium2 Bass kernel for nn_Disentangler (gnn_message_passing).

Math (per timestamp t, fully data-parallel across 8 cores):
  xn   = LN(x[t, :8192], ln1_g, ln1_b)
  tee  = scatter_add(xn by indices[t]) into 32768 slots
  h    = gelu(tee @ w1 + b1) @ w2 + b2
  comp = LNf(chunk_sum(h))                       # 16 chunks of 2048 slots
  dec  = (gelu(LNd(comp_rows) @ dw1 + db1) @ dw2 + db2)   # only 16 distinct rows
  out[t, i] = dec[indices[t, i] >> 11]; out[t, 8192:] = 0

Key transforms vs the reference:
  * LN folds into the first matmul: a_i = r_i * ((x_i - m_i) @ (g*w1)) [+ b@w1]
  * scatter-add runs at the 128-dim `a` level on *unique* slots only, via
    multiplicity-ordered dma_gather rounds from a DRAM spill of `a`.
  * chunk-sum (incl. empty-slot correction) is a matmul against a host-built
    per-chunk membership matrix M^T.
  * decode MLP computed on 16 rows; the final gather is a one-hot matmul with
    a split-bf16 (hi+lo) trick for full fp32 precision.
"""

import math

import numpy as np
import ml_dtypes

import concourse.bacc as bacc
import concourse.mybir as mybir
import concourse.tile as tile
import concourse.bass_utils as bass_utils
from concourse.masks import make_identity

# problem constants
T, NUM_TOKENS, D = 8, 12288, 256
N_NODE, NUM_NODES = 8192, 32768
L, C, H = 16, 64, 128          # COMP_LEN, COMP_DIM, 2*COMP_DIM
CHUNK = NUM_NODES // L         # 2048
P = 128
B = 16                         # token blocks per bigtile (2048 tokens / bigtile)
NBT = N_NODE // (P * B)        # 4 bigtiles
ZROW = N_NODE                  # zero row in the a-spill table
EPS = 1e-5
F32 = mybir.dt.float32
BF16 = mybir.dt.bfloat16
I16 = mybir.dt.int16
AF = mybir.ActivationFunctionType
ALU = mybir.AluOpType
SINGLE_PACKET = False


def _pack16(v):
    """int index list (len % 16 == 0) -> [128, n/16] int16 (wrap 16, replicate)."""
    a = np.asarray(v, np.int16).reshape(-1, 16).T
    return np.ascontiguousarray(np.tile(a, (8, 1)))


def blob_layout(cfg):
    """fp32 weight blob layout: name -> (row0, nrows, col0, ncols)."""
    NB = cfg["NB"]
    lay, col = {}, [0]

    def put(name, rows, cols):
        lay[name] = (0, rows, col[0], cols)
        col[0] += cols

    put("w1g0", P, H)
    put("w1g1", P, H)
    put("mt", P, NB * L)
    put("w2", H, C)
    put("dw1", C, H)
    put("dw2", H, D)
    put("b2r", 1, C)
    put("db1r", 1, H)
    put("db2r", 1, D)
    if not cfg["lnf_triv"]:
        put("lnfg", C, L)
        put("lnfb", C, L)
    if not cfg["lnd_triv"]:
        put("lndg", L, C)
        put("lndb", L, C)
    if cfg["has_bw1"]:
        put("bw1rep", P, H)
    if cfg["has_b1"]:
        put("b1rep", P, H)
    return lay, col[0]


def host_prep(x, indices, ln1_g, ln1_b, w1, b1, w2, b2,
              lnf_g, lnf_b, lnd_g, lnd_b, dw1, db1, dw2, db2):
    """Build per-core in_maps + global config."""
    f = np.float32
    x = np.asarray(x, f)
    ln1_g, ln1_b = np.asarray(ln1_g, f), np.asarray(ln1_b, f)
    w1, b1 = np.asarray(w1, f), np.asarray(b1, f)
    w2, b2 = np.asarray(w2, f), np.asarray(b2, f)
    lnf_g, lnf_b = np.asarray(lnf_g, f), np.asarray(lnf_b, f)
    lnd_g, lnd_b = np.asarray(lnd_g, f), np.asarray(lnd_b, f)
    dw1, db1 = np.asarray(dw1, f), np.asarray(db1, f)
    dw2, db2 = np.asarray(dw2, f), np.asarray(db2, f)

    per_t = []
    for t in range(T):
        idx = np.asarray(indices[t], np.int64)
        uniq, counts = np.unique(idx, return_counts=True)
        order = np.argsort(-counts, kind="stable")
        sp = np.argsort(idx, kind="stable")
        starts = np.zeros(uniq.size + 1, np.int64)
        starts[1:] = np.cumsum(counts)
        per_t.append(dict(idx=idx, uniq=uniq, counts=counts, order=order,
                          sp=sp, starts=starts, U=uniq.size,
                          K=int(counts.max())))

    U_max = max(d["U"] for d in per_t)
    U_pad = P * math.ceil((U_max + L) / P)
    NB = U_pad // P
    K_g = max(d["K"] for d in per_t)
    mks = []
    for k in range(1, K_g):
        mk = max(int((d["counts"] > k).sum()) for d in per_t)
        mks.append(P * math.ceil(mk / P))

    cfg = dict(
        U_pad=U_pad, NB=NB, K_g=K_g, mks=mks,
        has_bw1=bool(np.any(ln1_b != 0)),
        has_b1=bool(np.any(b1 != 0)),
        lnf_triv=bool(np.all(lnf_g == 1) and np.all(lnf_b == 0)),
        lnd_triv=bool(np.all(lnd_g == 1) and np.all(lnd_b == 0)),
    )
    lay, wcols = blob_layout(cfg)
    cfg["wcols"] = wcols
    cfg["idx_cols"] = (U_pad + sum(mks)) // 16

    W1g = ln1_g[:, None] * w1
    blob = np.zeros((P, wcols), np.float32)

    def put(name, val):
        r0, nr, c0, ncs = lay[name]
        blob[r0:r0 + nr, c0:c0 + ncs] = val

    put("w1g0", W1g[:P, :])
    put("w1g1", W1g[P:, :])
    put("w2", w2)
    put("dw1", dw1)
    put("dw2", dw2)
    put("b2r", (CHUNK * b2)[None, :])
    put("db1r", db1[None, :])
    put("db2r", db2[None, :])
    if not cfg["lnf_triv"]:
        put("lnfg", lnf_g.reshape(L, C).T)
        put("lnfb", lnf_b.reshape(L, C).T)
    if not cfg["lnd_triv"]:
        put("lndg", np.tile(lnd_g, (L, 1)))
        put("lndb", np.tile(lnd_b, (L, 1)))
    if cfg["has_bw1"]:
        put("bw1rep", np.tile((ln1_b @ w1)[None, :], (P, 1)))
    if cfg["has_b1"]:
        put("b1rep", np.tile(b1[None, :], (P, 1)))

    in_maps = []
    for t in range(T):
        d = per_t[t]
        idx, uniq, counts, order = d["idx"], d["uniq"], d["counts"], d["order"]
        sp, starts, U = d["sp"], d["starts"], d["U"]

        g0 = np.full(U_pad, ZROW, np.int64)
        g0[:U] = sp[starts[order]]
        gks = []
        for k in range(1, K_g):
            gk = np.full(mks[k - 1], ZROW, np.int64)
            sel = counts[order] > k          # prefix (desc multiplicity sort)
            nsel = int(sel.sum())
            if nsel:
                gk[:nsel] = sp[starts[order[sel]] + k]
            gks.append(gk)

        wblob = blob.copy()
        mt = np.zeros((U_pad, L), np.float32)
        lu = (uniq >> 11).astype(np.int64)
        mt[np.arange(U), lu[order]] = 1.0
        cnt_chunk = np.bincount(lu, minlength=L).astype(np.float32)
        mt[U + np.arange(L), np.arange(L)] = CHUNK - cnt_chunk
        r0, nr, c0, ncs = lay["mt"]
        wblob[r0:r0 + nr, c0:c0 + ncs] = \
            mt.reshape(NB, P, L).transpose(1, 0, 2).reshape(P, NB * L)

        l_arr = (idx >> 11).astype(np.int64)
        lv = l_arr.reshape(NBT, P, B)                  # token = bt*P*B + p*B + b
        oh = np.zeros((4 * L, NBT, B, P), np.float32)
        bt_i, p_i, b_i = np.indices((NBT, P, B))
        oh[lv, bt_i, b_i, p_i] = 1.0
        oh[lv + 2 * L, bt_i, b_i, p_i] = 1.0
        oh_dev = oh.reshape(4 * L, N_NODE).astype(ml_dtypes.bfloat16)

        iblob = np.concatenate([_pack16(g0)] + [_pack16(g) for g in gks], axis=1)
        in_maps.append({
            "xt": np.ascontiguousarray(x[t, :N_NODE, :]),
            "oh": oh_dev,
            "iblob": np.ascontiguousarray(iblob),
            "wblob": wblob,
        })
    return cfg, in_maps


def build(cfg, loop_k=0, phase='all'):
    """Build the Bass program. loop_k>0 wraps the body in a hardware loop
    (for timing); loop_k=0 emits a single-shot kernel."""
    U_pad, NB, K_g, mks = cfg["U_pad"], cfg["NB"], cfg["K_g"], cfg["mks"]
    lay, wcols = blob_layout(cfg)
    nc = bacc.Bacc("TRN2", num_devices=8)

    xt = nc.dram_tensor("xt", [N_NODE, D], F32, kind="ExternalInput").ap()
    oh_d = nc.dram_tensor("oh", [4 * L, N_NODE], BF16, kind="ExternalInput").ap()
    ib_d = nc.dram_tensor("iblob", [P, cfg["idx_cols"]], I16,
                          kind="ExternalInput").ap()
    wb_d = nc.dram_tensor("wblob", [P, wcols], F32, kind="ExternalInput").ap()
    out_d = nc.dram_tensor("out", [NUM_TOKENS, D], F32, kind="ExternalOutput").ap()
    adram = nc.dram_tensor("adram", [N_NODE + 1, H], F32, kind="Internal").ap()

    TPB = P * B  # tokens per bigtile

    with tile.TileContext(nc) as tc:
        with (
            tc.tile_pool(name="const", bufs=1) as cpool,
            tc.tile_pool(name="x", bufs=2) as xpool,
            tc.tile_pool(name="stats", bufs=2) as spool,
            tc.tile_pool(name="xT", bufs=3) as xtpool,
            tc.tile_pool(name="a", bufs=2) as apool,
            tc.tile_pool(name="acc", bufs=1) as accpool,
            tc.tile_pool(name="stg", bufs=2) as stgpool,
            tc.tile_pool(name="dec", bufs=1) as dpool,
            tc.tile_pool(name="outp", bufs=2) as opool,
            tc.tile_pool(name="ps_tr", bufs=2, space="PSUM") as ps_tr,
            tc.tile_pool(name="ps_mm", bufs=2, space="PSUM") as ps_mm,
            tc.tile_pool(name="ps_cs", bufs=1, space="PSUM") as ps_cs,
            tc.tile_pool(name="ps_out", bufs=2, space="PSUM") as ps_out,
            tc.tile_pool(name="ps_sm", bufs=1, space="PSUM") as ps_sm,
        ):
            # ---------- constants ----------
            ident = cpool.tile([P, P], F32)
            make_identity(nc, ident[:])
            zt = cpool.tile([P, 4096], F32)
            nc.vector.memset(zt[:], 0.0)
            ones16 = cpool.tile([1, L], F32)
            nc.vector.memset(ones16[:], 1.0)
            onescol = cpool.tile([C, 1], F32)
            nc.vector.memset(onescol[:], 1.0)

            wb = cpool.tile([P, wcols], F32)
            nc.sync.dma_start(out=wb[:], in_=wb_d[:])

            def w(name):
                r0, nr, c0, ncs = lay[name]
                return wb[r0:r0 + nr, c0:c0 + ncs]

            mt_sb = w("mt").rearrange("p (nb l) -> p nb l", l=L)
            oh_sb = cpool.tile([4 * L, N_NODE], BF16)
            nc.sync.dma_start(out=oh_sb[:], in_=oh_d[:])
            ib = cpool.tile([P, cfg["idx_cols"]], I16)
            nc.sync.dma_start(out=ib[:], in_=ib_d[:])
            ioffs = [0, U_pad // 16]
            for k in range(1, K_g):
                ioffs.append(ioffs[-1] + mks[k - 1] // 16)
            # zero row of the a-table
            nc.sync.dma_start(out=adram[ZROW:ZROW + 1, :], in_=zt[0:1, 0:H])

            def body(_i=None):
                # ---------- encode: LN-folded matmul + spill ----------
                for bt in range(NBT):
                    xb = xpool.tile([P, B, D], F32, tag="xb")
                    nc.sync.dma_start(
                        out=xb[:], in_=xt[bt * TPB:(bt + 1) * TPB, :])
                    st = spool.tile([P, B, 6], F32, tag="st")
                    mv = spool.tile([P, B, 2], F32, tag="mv")
                    for b in range(B):
                        nc.vector.bn_stats(st[:, b, :], xb[:, b, :])
                    for b in range(B):
                        nc.vector.bn_aggr(mv[:, b, :], st[:, b, :])
                    rc = spool.tile([P, B], F32, tag="rc")
                    nm = spool.tile([P, B], F32, tag="nm")
                    nc.vector.tensor_scalar_add(rc[:], mv[:, :, 1], EPS)
                    nc.scalar.sqrt(rc[:], rc[:])
                    nc.vector.reciprocal(rc[:], rc[:])
                    nc.vector.tensor_scalar_mul(nm[:], mv[:, :, 0], -1.0)
                    a_big = apool.tile([P, B, H], F32, tag="a")
                    for b in range(B):
                        nc.scalar.activation(
                            xb[:, b, :], xb[:, b, :], AF.Identity,
                            bias=nm[:, b:b + 1])
                        trp = ps_tr.tile([P, D], F32, space="PSUM", tag="trp")
                        nc.tensor.transpose(
                            out=trp[:, 0:P], in_=xb[:, b, 0:P], identity=ident[:])
                        nc.tensor.transpose(
                            out=trp[:, P:D], in_=xb[:, b, P:D], identity=ident[:])
                        xTs = xtpool.tile([P, D], F32, tag="xT")
                        nc.vector.tensor_copy(out=xTs[:], in_=trp[:])
                        pp = ps_mm.tile([P, H], F32, space="PSUM", tag="pp")
                        nc.tensor.matmul(out=pp[:], lhsT=xTs[:, 0:P],
                                         rhs=w("w1g0"), start=True, stop=False)
                        nc.tensor.matmul(out=pp[:], lhsT=xTs[:, P:D],
                                         rhs=w("w1g1"), start=False, stop=True)
                        nc.scalar.activation(
                            a_big[:, b, :], pp[:], AF.Identity,
                            scale=rc[:, b:b + 1])
                        if cfg["has_bw1"]:
                            nc.vector.tensor_tensor(
                                out=a_big[:, b, :], in0=a_big[:, b, :],
                                in1=w("bw1rep"), op=ALU.add)
                    nc.sync.dma_start(
                        out=adram[bt * TPB:(bt + 1) * TPB, :], in_=a_big[:])

                if phase == 'spill':
                    for z in range(6):
                        nc.scalar.dma_start(
                            out=out_d[z * 2048:(z + 1) * 2048, :], in_=zt[:])
                    return

                # ---------- gather-accumulate over unique slots ----------
                acc = accpool.tile([P, NB, H], F32, tag="acc")
                nc.gpsimd.dma_gather(
                    acc[:], adram[:], ib[:, ioffs[0]:ioffs[1]], U_pad, U_pad, H,
                    single_packet=SINGLE_PACKET)
                if K_g > 1:
                    nbk = mks[0] // P
                    stg = stgpool.tile([P, nbk, H], F32, tag="stg")
                    nc.gpsimd.dma_gather(
                        stg[:], adram[:], ib[:, ioffs[1]:ioffs[2]], mks[0],
                        mks[0], H, single_packet=SINGLE_PACKET)
                    nc.vector.tensor_tensor(
                        out=acc[:, 0:nbk, :], in0=acc[:, 0:nbk, :], in1=stg[:],
                        op=ALU.add)
                if K_g > 2:
                    ntail = sum(mks[1:])
                    nbt2 = ntail // P
                    stg2 = stgpool.tile([P, nbt2, H], F32, tag="stg2")
                    nc.gpsimd.dma_gather(
                        stg2[:], adram[:], ib[:, ioffs[2]:ioffs[2] + ntail // 16],
                        ntail, ntail, H, single_packet=SINGLE_PACKET)
                    off = 0
                    for k in range(2, K_g):
                        nbk = mks[k - 1] // P
                        nc.vector.tensor_tensor(
                            out=acc[:, 0:nbk, :], in0=acc[:, 0:nbk, :],
                            in1=stg2[:, off:off + nbk, :], op=ALU.add)
                        off += nbk
                if cfg["has_b1"]:
                    for blk in range(NB):
                        nc.vector.tensor_tensor(
                            out=acc[:, blk, :], in0=acc[:, blk, :],
                            in1=w("b1rep"), op=ALU.add)
                for blk0 in range(0, NB, 8):
                    blk1 = min(blk0 + 8, NB)
                    nc.scalar.activation(
                        acc[:, blk0:blk1, :], acc[:, blk0:blk1, :], AF.Gelu)

                if phase == 'gather':
                    nc.scalar.dma_start(out=out_d[0:NB * 64, :], in_=acc[:])
                    for z in range(2):
                        nc.scalar.dma_start(
                            out=out_d[N_NODE + z * 2048:N_NODE + (z + 1) * 2048, :],
                            in_=zt[:])
                    return

                # ---------- chunk-sum matmul + w2 ----------
                cps = ps_cs.tile([P, L], F32, space="PSUM", tag="cps")
                for blk in range(NB):
                    nc.tensor.matmul(out=cps[:], lhsT=acc[:, blk, :],
                                     rhs=mt_sb[:, blk, :],
                                     start=(blk == 0), stop=(blk == NB - 1))
                compT = dpool.tile([P, L], F32, tag="compT")
                nc.vector.tensor_copy(out=compT[:], in_=cps[:])
                c2ps = ps_sm.tile([C, L], F32, space="PSUM", tag="sm")
                nc.tensor.matmul(out=c2ps[:], lhsT=w("w2"), rhs=compT[:],
                                 start=True, stop=False)
                nc.tensor.matmul(out=c2ps[:], lhsT=w("b2r"), rhs=ones16[:],
                                 start=False, stop=True)
                c2 = dpool.tile([C, L], F32, tag="c2")
                nc.vector.tensor_copy(out=c2[:], in_=c2ps[:])

                # ---------- LNf over the flattened [16*64] ----------
                junk = dpool.tile([C, L], F32, tag="junk")
                rs = dpool.tile([C, 1], F32, tag="rs")
                sqs = dpool.tile([C, 1], F32, tag="sqs")
                nc.scalar.activation(junk[:], c2[:], AF.Identity, accum_out=rs[:])
                nc.scalar.activation(junk[:], c2[:], AF.Square, accum_out=sqs[:])
                t1ps = ps_sm.tile([1, 1], F32, space="PSUM", tag="sm")
                t2ps = ps_sm.tile([1, 1], F32, space="PSUM", tag="sm")
                nc.tensor.matmul(out=t1ps[:], lhsT=rs[:], rhs=onescol[:],
                                 start=True, stop=True)
                nc.tensor.matmul(out=t2ps[:], lhsT=sqs[:], rhs=onescol[:],
                                 start=True, stop=True)
                mean = dpool.tile([1, 1], F32, tag="mean")
                msq = dpool.tile([1, 1], F32, tag="msq")
                nc.vector.tensor_scalar_mul(mean[:], t1ps[:], 1.0 / (L * C))
                nc.vector.tensor_scalar_mul(msq[:], t2ps[:], 1.0 / (L * C))
                var = dpool.tile([1, 1], F32, tag="var")
                nc.vector.tensor_tensor(out=var[:], in0=mean[:], in1=mean[:],
                                        op=ALU.mult)
                nc.vector.tensor_tensor(out=var[:], in0=msq[:], in1=var[:],
                                        op=ALU.subtract)
                rstd = dpool.tile([1, 1], F32, tag="rstd")
                nc.vector.tensor_scalar_add(rstd[:], var[:], EPS)
                nc.scalar.sqrt(rstd[:], rstd[:])
                nc.vector.reciprocal(rstd[:], rstd[:])
                nmr = dpool.tile([1, 1], F32, tag="nmr")
                nc.vector.tensor_scalar(out=nmr[:], in0=mean[:], scalar1=rstd[:],
                                        scalar2=-1.0, op0=ALU.mult, op1=ALU.mult)
                bc_r = dpool.tile([C, 1], F32, tag="bc_r")
                bc_n = dpool.tile([C, 1], F32, tag="bc_n")
                nc.gpsimd.partition_broadcast(bc_r[:], rstd[:])
                nc.gpsimd.partition_broadcast(bc_n[:], nmr[:])
                c2n = dpool.tile([C, L], F32, tag="c2n")
                nc.scalar.activation(c2n[:], c2[:], AF.Identity,
                                     bias=bc_n[:], scale=bc_r[:])
                if not cfg["lnf_triv"]:
                    nc.vector.tensor_tensor(out=c2n[:], in0=c2n[:],
                                            in1=w("lnfg"), op=ALU.mult)
                    nc.vector.tensor_tensor(out=c2n[:], in0=c2n[:],
                                            in1=w("lnfb"), op=ALU.add)

                # ---------- LNd per row + decode MLP (tiny) ----------
                cfps = ps_sm.tile([L, C], F32, space="PSUM", tag="sm")
                nc.tensor.transpose(out=cfps[:], in_=c2n[:], identity=ident[0:C, 0:C])
                cf = dpool.tile([L, C], F32, tag="cf")
                nc.vector.tensor_copy(out=cf[:], in_=cfps[:])
                st2 = dpool.tile([L, 6], F32, tag="st2")
                mv2 = dpool.tile([L, 2], F32, tag="mv2")
                nc.vector.bn_stats(st2[:], cf[:])
                nc.vector.bn_aggr(mv2[:], st2[:])
                rc2 = dpool.tile([L, 1], F32, tag="rc2")
                nm2 = dpool.tile([L, 1], F32, tag="nm2")
                nc.vector.tensor_scalar_add(rc2[:], mv2[:, 1:2], EPS)
                nc.scalar.sqrt(rc2[:], rc2[:])
                nc.vector.reciprocal(rc2[:], rc2[:])
                nc.vector.tensor_scalar(out=nm2[:], in0=mv2[:, 0:1], scalar1=rc2[:],
                                        scalar2=-1.0, op0=ALU.mult, op1=ALU.mult)
                t2n = dpool.tile([L, C], F32, tag="t2n")
                nc.scalar.activation(t2n[:], cf[:], AF.Identity,
                                     bias=nm2[:], scale=rc2[:])
                if not cfg["lnd_triv"]:
                    nc.vector.tensor_tensor(out=t2n[:], in0=t2n[:],
                                            in1=w("lndg"), op=ALU.mult)
                    nc.vector.tensor_tensor(out=t2n[:], in0=t2n[:],
                                            in1=w("lndb"), op=ALU.add)
                ttps = ps_sm.tile([C, L], F32, space="PSUM", tag="sm")
                nc.tensor.transpose(out=ttps[:], in_=t2n[:], identity=ident[0:L, 0:L])
                t2nT = dpool.tile([C, L], F32, tag="t2nT")
                nc.vector.tensor_copy(out=t2nT[:], in_=ttps[:])

                d1ps = ps_mm.tile([P, L], F32, space="PSUM", tag="pp")
                nc.tensor.matmul(out=d1ps[:], lhsT=w("dw1"), rhs=t2nT[:],
                                 start=True, stop=False)
                nc.tensor.matmul(out=d1ps[:], lhsT=w("db1r"), rhs=ones16[:],
                                 start=False, stop=True)
                d1T = dpool.tile([P, L], F32, tag="d1T")
                nc.scalar.activation(d1T[:], d1ps[:], AF.Gelu)
                decps = ps_out.tile([L, D], F32, space="PSUM", tag="ops")
                nc.tensor.matmul(out=decps[:], lhsT=d1T[:], rhs=w("dw2"),
                                 start=True, stop=False)
                nc.tensor.matmul(out=decps[:], lhsT=ones16[:], rhs=w("db2r"),
                                 start=False, stop=True)
                dec = dpool.tile([L, D], F32, tag="dec")
                nc.vector.tensor_copy(out=dec[:], in_=decps[:])
                dhl = dpool.tile([4 * L, D], BF16, tag="dhl")
                nc.vector.memset(dhl[:], 0.0)
                nc.vector.tensor_copy(out=dhl[0:L, :], in_=dec[:])
                dhi32 = dpool.tile([L, D], F32, tag="dhi32")
                nc.vector.tensor_copy(out=dhi32[:], in_=dhl[0:L, :])
                dlo = dpool.tile([L, D], F32, tag="dlo")
                nc.vector.tensor_tensor(out=dlo[:], in0=dec[:], in1=dhi32[:],
                                        op=ALU.subtract)
                nc.vector.tensor_copy(out=dhl[2 * L:3 * L, :], in_=dlo[:])

                if phase == 'dec':
                    nc.scalar.dma_start(out=out_d[0:64, :],
                                        in_=dhl[:].bitcast(F32))
                    for z in range(2):
                        nc.scalar.dma_start(
                            out=out_d[N_NODE + z * 2048:N_NODE + (z + 1) * 2048, :],
                            in_=zt[:])
                    return

                # ---------- output gather (one-hot matmul) + zeros ----------
                for bt in range(NBT):
                    ob = opool.tile([P, B, D], F32, tag="ob")
                    for b in range(B):
                        col = (bt * B + b) * P
                        ops_ = ps_out.tile([P, D], F32, space="PSUM", tag="ops")
                        nc.tensor.matmul(out=ops_[:], lhsT=oh_sb[:, col:col + P],
                                         rhs=dhl[:], start=True, stop=True)
                        if b % 2 == 0:
                            nc.vector.tensor_copy(out=ob[:, b, :], in_=ops_[:])
                        else:
                            nc.scalar.copy(out=ob[:, b, :], in_=ops_[:])
                    nc.scalar.dma_start(
                        out=out_d[bt * TPB:(bt + 1) * TPB, :], in_=ob[:])
                for z in range(2):
                    nc.scalar.dma_start(
                        out=out_d[N_NODE + z * 2048:N_NODE + (z + 1) * 2048, :],
                        in_=zt[:])

            if loop_k > 0:
                with tc.For_i(0, loop_k, 1,
                              hint_engines=(mybir.EngineType.PE,
                                            mybir.EngineType.DVE,
                                            mybir.EngineType.Activation,
                                            mybir.EngineType.Pool,
                                            mybir.EngineType.SP)):
                    body()
            else:
                body()

    nc.compile()
    return nc


def kernel(**inputs) -> np.ndarray:
    cfg, in_maps = host_prep(**inputs)
    nc = build(cfg)
    res = bass_utils.run_bass_kernel_spmd(nc, in_maps, core_ids=list(range(T)))
    out = np.stack([res.results[c]["out"] for c in range(T)], axis=0)
    return out.astype(np.float32)


# revision 23
# speedup vs baseline: 1.7670x; 1.7670x over previous
"""Trainium2 Bass kernel for nn_Disentangler (gnn_message_passing).

Math (per timestamp t, fully data-parallel across 8 cores):
  xn   = LN(x[t, :8192], ln1_g, ln1_b)
  tee  = scatter_add(xn by indices[t]) into 32768 slots
  h    = gelu(tee @ w1 + b1) @ w2 + b2
  comp = LNf(chunk_sum(h))                       # 16 chunks of 2048 slots
  dec  = (gelu(LNd(comp_rows) @ dw1 + db1) @ dw2 + db2)   # only 16 distinct rows
  out[t, i] = dec[indices[t, i] >> 11]; out[t, 8192:] = 0

Key transforms vs the reference:
  * LN folds into the first matmul: a_i = r_i * ((x_i - m_i) @ (g*w1)) [+ b@w1]
  * slots hit by exactly one token need no scatter at all: their gelu(a_i)
    feeds the chunk-sum matmul directly in token order (M_tok).
  * only multi-hit slots (~12%) are accumulated, via multiplicity-ordered
    dma_gather rounds from a bf16 DRAM spill of `a` (M_mul matmul after gelu).
  * chunk-sum (incl. empty-slot correction via weighted fake rows) is a bf16
    matmul against host-built membership matrices.
  * decode MLP computed on 16 rows; the final gather is a one-hot matmul with
    a split-bf16 (hi+lo) trick for full fp32 precision.
"""

import math

import numpy as np
import ml_dtypes

import concourse.bacc as bacc
import concourse.mybir as mybir
import concourse.tile as tile
import concourse.bass_utils as bass_utils
from concourse.masks import make_identity

# problem constants
T, NUM_TOKENS, D = 8, 12288, 256
N_NODE, NUM_NODES = 8192, 32768
L, C, H = 16, 64, 128          # COMP_LEN, COMP_DIM, 2*COMP_DIM
CHUNK = NUM_NODES // L         # 2048
P = 128
B = 16                         # token blocks per bigtile (2048 tokens / bigtile)
NBT = N_NODE // (P * B)        # 4 bigtiles
NBLK = N_NODE // P             # 64 token blocks
ZROW = N_NODE                  # zero row in the a-spill table
EPS = 1e-5
F32 = mybir.dt.float32
BF16 = mybir.dt.bfloat16
I16 = mybir.dt.int16
AF = mybir.ActivationFunctionType
ALU = mybir.AluOpType
SINGLE_PACKET = False
COPY_ENG = 'mix'


def _pack16(v):
    """int index list (len % 16 == 0) -> [128, n/16] int16 (wrap 16, replicate)."""
    a = np.asarray(v, np.int16).reshape(-1, 16).T
    return np.ascontiguousarray(np.tile(a, (8, 1)))


def blob_layout(cfg):
    """fp32 weight blob layout: name -> (row0, nrows, col0, ncols)."""
    lay, col = {}, [0]

    def put(name, rows, cols):
        lay[name] = (0, rows, col[0], cols)
        col[0] += cols

    put("w1g0", P, H)
    put("w1g1", P, H)
    put("w2", H, C)
    put("dw1", C, H)
    put("dw2", H, D)
    put("b2r", 1, C)
    put("db1r", 1, H)
    put("db2r", 1, D)
    if not cfg["lnf_triv"]:
        put("lnfg", C, L)
        put("lnfb", C, L)
    if not cfg["lnd_triv"]:
        put("lndg", L, C)
        put("lndb", L, C)
    if cfg["has_bw1"]:
        put("bw1rep", P, H)
    if cfg["has_b1"]:
        put("b1rep", P, H)
    return lay, col[0]


def iblob_layout(cfg):
    """int16 blob: gather tables + bf16 membership matrices (bitcast)."""
    NBm, mks = cfg["NBm"], cfg["mks"]
    lay, col = {}, [0]

    def put(name, cols):
        lay[name] = (col[0], cols)
        col[0] += cols

    put("g0", cfg["Um_pad"] // 16)
    if cfg["K_g"] > 1:
        put("g1", mks[0] // 16)
    if cfg["K_g"] > 2:
        put("gt", sum(mks[1:]) // 16)
    put("mtok", NBLK * L)        # [128, 64*16] bf16
    put("mtm", NBm * L)          # [128, NBm*16] bf16
    put("w1gb0", H)              # [128, 128] bf16
    put("w1gb1", H)
    return lay, col[0]


def host_prep(x, indices, ln1_g, ln1_b, w1, b1, w2, b2,
              lnf_g, lnf_b, lnd_g, lnd_b, dw1, db1, dw2, db2):
    """Build per-core in_maps + global config."""
    f = np.float32
    x = np.asarray(x, f)
    ln1_g, ln1_b = np.asarray(ln1_g, f), np.asarray(ln1_b, f)
    w1, b1 = np.asarray(w1, f), np.asarray(b1, f)
    w2, b2 = np.asarray(w2, f), np.asarray(b2, f)
    lnf_g, lnf_b = np.asarray(lnf_g, f), np.asarray(lnf_b, f)
    lnd_g, lnd_b = np.asarray(lnd_g, f), np.asarray(lnd_b, f)
    dw1, db1 = np.asarray(dw1, f), np.asarray(db1, f)
    dw2, db2 = np.asarray(dw2, f), np.asarray(db2, f)

    per_t = []
    for t in range(T):
        idx = np.asarray(indices[t], np.int64)
        uniq, counts = np.unique(idx, return_counts=True)
        order = np.argsort(-counts, kind="stable")   # multi slots first
        sp = np.argsort(idx, kind="stable")
        starts = np.zeros(uniq.size + 1, np.int64)
        starts[1:] = np.cumsum(counts)
        per_t.append(dict(idx=idx, uniq=uniq, counts=counts, order=order,
                          sp=sp, starts=starts,
                          M=int((counts >= 2).sum()), K=int(counts.max())))

    K_g = max(d["K"] for d in per_t)
    M_max = max(d["M"] for d in per_t)
    Um_pad = P * math.ceil((M_max + L) / P)
    NBm = Um_pad // P
    mks = []
    for k in range(1, K_g):
        mk = max(int((d["counts"] > k).sum()) for d in per_t)
        mks.append(P * math.ceil(mk / P))

    cfg = dict(
        Um_pad=Um_pad, NBm=NBm, K_g=K_g, mks=mks,
        has_bw1=bool(np.any(ln1_b != 0)),
        has_b1=bool(np.any(b1 != 0)),
        lnf_triv=bool(np.all(lnf_g == 1) and np.all(lnf_b == 0)),
        lnd_triv=bool(np.all(lnd_g == 1) and np.all(lnd_b == 0)),
    )
    lay, wcols = blob_layout(cfg)
    ilay, icols = iblob_layout(cfg)
    cfg["wcols"], cfg["icols"] = wcols, icols

    W1g = ln1_g[:, None] * w1
    blob = np.zeros((P, wcols), np.float32)

    def put(name, val):
        r0, nr, c0, ncs = lay[name]
        blob[r0:r0 + nr, c0:c0 + ncs] = val

    put("w1g0", W1g[:P, :])
    put("w1g1", W1g[P:, :])
    put("w2", w2)
    put("dw1", dw1)
    put("dw2", dw2)
    put("b2r", (CHUNK * b2)[None, :])
    put("db1r", db1[None, :])
    put("db2r", db2[None, :])
    if not cfg["lnf_triv"]:
        put("lnfg", lnf_g.reshape(L, C).T)
        put("lnfb", lnf_b.reshape(L, C).T)
    if not cfg["lnd_triv"]:
        put("lndg", np.tile(lnd_g, (L, 1)))
        put("lndb", np.tile(lnd_b, (L, 1)))
    if cfg["has_bw1"]:
        put("bw1rep", np.tile((ln1_b @ w1)[None, :], (P, 1)))
    if cfg["has_b1"]:
        put("b1rep", np.tile(b1[None, :], (P, 1)))

    in_maps = []
    for t in range(T):
        d = per_t[t]
        idx, uniq, counts, order = d["idx"], d["uniq"], d["counts"], d["order"]
        sp, starts, M = d["sp"], d["starts"], d["M"]

        # gather tables (multi-hit slots only; desc-multiplicity prefix order)
        g0 = np.full(Um_pad, ZROW, np.int64)
        g0[:M] = sp[starts[order[:M]]]
        gks = []
        for k in range(1, K_g):
            gk = np.full(mks[k - 1], ZROW, np.int64)
            sel = counts[order] > k
            nsel = int(sel.sum())
            if nsel:
                gk[:nsel] = sp[starts[order[sel]] + k]
            gks.append(gk)

        # M_mul: multi-compact rows -> chunk, plus empty-correction fakes
        lu = (uniq >> 11).astype(np.int64)
        mtm = np.zeros((Um_pad, L), np.float32)
        mtm[np.arange(M), lu[order[:M]]] = 1.0
        cnt_chunk = np.bincount(lu, minlength=L).astype(np.float32)
        mtm[M + np.arange(L), np.arange(L)] = CHUNK - cnt_chunk
        mtm_dev = mtm.reshape(NBm, P, L).transpose(1, 0, 2).reshape(P, NBm * L)

        # M_tok: singleton-slot tokens -> chunk, in token order
        mtok = np.zeros((N_NODE, L), np.float32)
        sing = counts == 1
        spos = sp[starts[:-1][sing]]              # the single occurrence
        mtok[spos, lu[sing]] = 1.0
        # token = bt*2048 + p*16 + b  -> dev [p, bt*16+b, l]
        mtok_dev = (mtok.reshape(NBT, P, B, L).transpose(1, 0, 2, 3)
                    .reshape(P, NBLK * L))

        # output staging writes half-bigtiles: token = ht*1024 + p*8 + b
        l_arr = (idx >> 11).astype(np.int64)
        HB = B // 2
        lv = l_arr.reshape(2 * NBT, P, HB)
        oh = np.zeros((4 * L, 2 * NBT, HB, P), np.float32)
        ht_i, p_i, b_i = np.indices((2 * NBT, P, HB))
        oh[lv, ht_i, b_i, p_i] = 1.0
        oh[lv + 2 * L, ht_i, b_i, p_i] = 1.0
        oh_dev = oh.reshape(4 * L, N_NODE).astype(ml_dtypes.bfloat16)

        iblob = np.zeros((P, icols), np.int16)

        def iput(name, val):
            c0, ncs = ilay[name]
            iblob[:, c0:c0 + ncs] = val

        iput("g0", _pack16(g0))
        if K_g > 1:
            iput("g1", _pack16(gks[0]))
        if K_g > 2:
            iput("gt", np.concatenate([_pack16(g) for g in gks[1:]], axis=1))
        iput("mtok", np.tile(mtok_dev.astype(ml_dtypes.bfloat16)
                             .view(np.int16), (1, 1)))
        iput("mtm", mtm_dev.astype(ml_dtypes.bfloat16).view(np.int16))
        iput("w1gb0", W1g[:P, :].astype(ml_dtypes.bfloat16).view(np.int16))
        iput("w1gb1", W1g[P:, :].astype(ml_dtypes.bfloat16).view(np.int16))

        in_maps.append({
            "xt": np.ascontiguousarray(x[t, :N_NODE, :]),
            "oh": oh_dev,
            "iblob": np.ascontiguousarray(iblob),
            "wblob": blob,
        })
    return cfg, in_maps


def build(cfg, loop_k=0, phase='all'):
    """Build the Bass program. loop_k>0 wraps the body in a hardware loop
    (for timing); loop_k=0 emits a single-shot kernel."""
    Um_pad, NBm, K_g, mks = cfg["Um_pad"], cfg["NBm"], cfg["K_g"], cfg["mks"]
    lay, wcols = blob_layout(cfg)
    ilay, icols = iblob_layout(cfg)
    nc = bacc.Bacc("TRN2", num_devices=8, num_swdge_queues=2)

    xt = nc.dram_tensor("xt", [N_NODE, D], F32, kind="ExternalInput").ap()
    oh_d = nc.dram_tensor("oh", [4 * L, N_NODE], BF16, kind="ExternalInput").ap()
    ib_d = nc.dram_tensor("iblob", [P, icols], I16, kind="ExternalInput").ap()
    wb_d = nc.dram_tensor("wblob", [P, wcols], F32, kind="ExternalInput").ap()
    out_d = nc.dram_tensor("out", [NUM_TOKENS, D], F32, kind="ExternalOutput").ap()
    adram = nc.dram_tensor("adram", [N_NODE + 1, H], BF16, kind="Internal").ap()

    TPB = P * B  # tokens per bigtile

    with tile.TileContext(nc) as tc:
        with (
            tc.tile_pool(name="const", bufs=1) as cpool,
            tc.tile_pool(name="x", bufs=2) as xpool,
            tc.tile_pool(name="xc", bufs=2) as xcpool,
            tc.tile_pool(name="stats", bufs=2) as spool,
            tc.tile_pool(name="xT", bufs=2) as xtpool,
            tc.tile_pool(name="a", bufs=1) as apool,
            tc.tile_pool(name="ga", bufs=1) as gapool,
            tc.tile_pool(name="acc", bufs=1) as accpool,
            tc.tile_pool(name="stg", bufs=2) as stgpool,
            tc.tile_pool(name="dec", bufs=1) as dpool,
            tc.tile_pool(name="outp", bufs=2) as opool,
            tc.tile_pool(name="ps_tr", bufs=2, space="PSUM") as ps_tr,
            tc.tile_pool(name="ps_mm", bufs=2, space="PSUM") as ps_mm,
            tc.tile_pool(name="ps_cs", bufs=1, space="PSUM") as ps_cs,
            tc.tile_pool(name="ps_out", bufs=2, space="PSUM") as ps_out,
            tc.tile_pool(name="ps_sm", bufs=1, space="PSUM") as ps_sm,
        ):
            # ---------- constants ----------
            ident = cpool.tile([P, P], F32)
            make_identity(nc, ident[:])
            identb = cpool.tile([P, P], BF16)
            nc.vector.tensor_copy(out=identb[:], in_=ident[:])
            zt = cpool.tile([P, 2048], F32)
            nc.vector.memset(zt[:], 0.0)
            ones16 = cpool.tile([1, L], F32)
            nc.vector.memset(ones16[:], 1.0)
            onescol = cpool.tile([C, 1], F32)
            nc.vector.memset(onescol[:], 1.0)

            wb = cpool.tile([P, wcols], F32)
            nc.sync.dma_start(out=wb[:], in_=wb_d[:])

            def w(name):
                r0, nr, c0, ncs = lay[name]
                return wb[r0:r0 + nr, c0:c0 + ncs]

            ib = cpool.tile([P, icols], I16)
            nc.sync.dma_start(out=ib[:], in_=ib_d[:])

            def iw(name):
                c0, ncs = ilay[name]
                return ib[:, c0:c0 + ncs]

            mtok_sb = iw("mtok").bitcast(BF16).rearrange(
                "p (nb l) -> p nb l", l=L)
            mtm_sb = iw("mtm").bitcast(BF16).rearrange(
                "p (nb l) -> p nb l", l=L)
            oh_sb = cpool.tile([4 * L, N_NODE], BF16)
            nc.sync.dma_start(out=oh_sb[:], in_=oh_d[:])
            # zero row of the bf16 a-table
            nc.sync.dma_start(out=adram[ZROW:ZROW + 1, :],
                              in_=zt[0:1, 0:C].bitcast(BF16))

            LVL = {'null': 0, 'xload': 1, 'ln': 2, 'cen': 3, 'tr': 4,
                   'mm': 5, 'spill': 6, 'gather': 7, 'dec': 8, 'all': 9}
            lvl = LVL[phase]

            def body(_i=None):
                if lvl == 0:
                    nc.scalar.dma_start(out=out_d[0:1024, :], in_=zt[:])
                    return
                # ---------- encode: LN-folded matmul + bf16 spill ----------
                a_sb = apool.tile([P, NBLK, H], F32, tag="a")
                ga = gapool.tile([P, NBLK, H], BF16, tag="ga")
                for bt in range(NBT):
                    xb = xpool.tile([P, B, D], F32, tag="xb")
                    nc.sync.dma_start(
                        out=xb[:], in_=xt[bt * TPB:(bt + 1) * TPB, :])
                    if lvl <= 1:
                        continue
                    st = spool.tile([P, B, 6], F32, tag="st")
                    mv = spool.tile([P, B, 2], F32, tag="mv")
                    for b in range(B):
                        nc.vector.bn_stats(st[:, b, :], xb[:, b, :])
                    for b in range(B):
                        nc.vector.bn_aggr(mv[:, b, :], st[:, b, :])
                    rc = spool.tile([P, B], F32, tag="rc")
                    nm = spool.tile([P, B], F32, tag="nm")
                    nc.vector.tensor_scalar_add(rc[:], mv[:, :, 1], EPS)
                    nc.scalar.sqrt(rc[:], rc[:])
                    nc.vector.reciprocal(rc[:], rc[:])
                    nc.vector.tensor_scalar_mul(nm[:], mv[:, :, 0], -1.0)
                    if lvl <= 2:
                        continue
                    xc = xcpool.tile([P, B, D], BF16, tag="xc")
                    for b in range(B):
                        nc.vector.tensor_scalar(
                            out=xc[:, b, :], in0=xb[:, b, :],
                            scalar1=nm[:, b:b + 1], scalar2=rc[:, b:b + 1],
                            op0=ALU.add, op1=ALU.mult)
                    if lvl <= 3:
                        continue
                    trps, pps = [], []
                    for b in range(B):
                        trp = ps_tr.tile([P, D], BF16, space="PSUM", tag="trp")
                        trps.append(trp)
                        nc.tensor.transpose(
                            out=trp[:, 0:P], in_=xc[:, b, 0:P], identity=identb[:])
                        nc.tensor.transpose(
                            out=trp[:, P:D], in_=xc[:, b, P:D], identity=identb[:])
                    xTb = xtpool.tile([P, B, D], BF16, tag="xT")
                    for b in range(B):
                        dve = COPY_ENG == 'dve' or (COPY_ENG == 'mix' and b % 2 == 0)
                        if dve:
                            nc.vector.tensor_copy(out=xTb[:, b, :], in_=trps[b][:])
                        else:
                            nc.scalar.copy(out=xTb[:, b, :], in_=trps[b][:])
                    if lvl <= 4:
                        continue
                    for b in range(B):
                        pp = ps_mm.tile([P, H], F32, space="PSUM", tag="pp")
                        pps.append(pp)
                        nc.tensor.matmul(out=pp[:], lhsT=xTb[:, b, 0:P],
                                         rhs=iw("w1gb0").bitcast(BF16),
                                         start=True, stop=False)
                        nc.tensor.matmul(out=pp[:], lhsT=xTb[:, b, P:D],
                                         rhs=iw("w1gb1").bitcast(BF16),
                                         start=False, stop=True)
                        dve = COPY_ENG == 'dve' or (COPY_ENG == 'mix' and b % 2)
                        if dve:
                            nc.vector.tensor_copy(out=a_sb[:, bt * B + b, :],
                                                  in_=pp[:])
                        else:
                            nc.scalar.copy(out=a_sb[:, bt * B + b, :], in_=pp[:])
                    if cfg["has_bw1"]:
                        for b in range(B):
                            blk = bt * B + b
                            nc.vector.tensor_tensor(
                                out=a_sb[:, blk, :], in0=a_sb[:, blk, :],
                                in1=w("bw1rep"), op=ALU.add)
                    if lvl <= 5:
                        continue
                    # bf16 spill for the multi-slot gather (SWDGE cast)
                    nc.gpsimd.dma_start(
                        out=adram[bt * TPB:(bt + 1) * TPB, :],
                        in_=a_sb[:, bt * B:(bt + 1) * B, :])
                if lvl <= 5:
                    nc.scalar.dma_start(out=out_d[0:1024, :], in_=zt[:])
                    return
                # gelu(a [+ b1]) for the singleton path, cast to bf16
                if cfg["has_b1"]:
                    for blk in range(NBLK):
                        nc.vector.tensor_tensor(
                            out=a_sb[:, blk, :], in0=a_sb[:, blk, :],
                            in1=w("b1rep"), op=ALU.add)
                for blk0 in range(0, NBLK, 8):
                    nc.scalar.activation(
                        ga[:, blk0:blk0 + 8, :], a_sb[:, blk0:blk0 + 8, :],
                        AF.Gelu)

                if phase == 'spill':
                    for z in range(4):
                        nc.scalar.dma_start(
                            out=out_d[z * 1024:(z + 1) * 1024, :], in_=zt[:])
                    return

                # ---------- gather-accumulate multi-hit slots ----------
                acc = accpool.tile([P, NBm, H], BF16, tag="acc")
                nc.gpsimd.dma_gather(
                    acc[:], adram[:], iw("g0"), Um_pad, Um_pad, H,
                    single_packet=SINGLE_PACKET, queue_num=0)
                if K_g > 1:
                    nbk = mks[0] // P
                    stg = stgpool.tile([P, nbk, H], BF16, tag="stg")
                    nc.gpsimd.dma_gather(
                        stg[:], adram[:], iw("g1"), mks[0], mks[0], H,
                        single_packet=SINGLE_PACKET, queue_num=1)
                    nc.vector.tensor_tensor(
                        out=acc[:, 0:nbk, :], in0=acc[:, 0:nbk, :], in1=stg[:],
                        op=ALU.add)
                if K_g > 2:
                    ntail = sum(mks[1:])
                    stg2 = stgpool.tile([P, ntail // P, H], BF16, tag="stg2")
                    nc.gpsimd.dma_gather(
                        stg2[:], adram[:], iw("gt"), ntail, ntail, H,
                        single_packet=SINGLE_PACKET, queue_num=0)
                    off = 0
                    for k in range(2, K_g):
                        nbk = mks[k - 1] // P
                        nc.vector.tensor_tensor(
                            out=acc[:, 0:nbk, :], in0=acc[:, 0:nbk, :],
                            in1=stg2[:, off:off + nbk, :], op=ALU.add)
                        off += nbk
                if cfg["has_b1"]:
                    for blk in range(NBm):
                        nc.vector.tensor_tensor(
                            out=acc[:, blk, :], in0=acc[:, blk, :],
                            in1=w("b1rep"), op=ALU.add)
                gm = accpool.tile([P, NBm, H], BF16, tag="gm")
                nc.scalar.activation(gm[:], acc[:], AF.Gelu)

                if phase == 'gather':
                    nc.scalar.dma_start(out=out_d[0:NBm * 32, :],
                                        in_=gm[:].bitcast(F32))
                    for z in range(4):
                        nc.scalar.dma_start(
                            out=out_d[N_NODE + z * 1024:N_NODE + (z + 1) * 1024, :],
                            in_=zt[:])
                    return

                # ---------- chunk-sum matmul (tokens + multi) + w2 ----------
                cps = ps_cs.tile([P, L], F32, space="PSUM", tag="cps")
                for blk in range(NBLK):
                    nc.tensor.matmul(out=cps[:], lhsT=ga[:, blk, :],
                                     rhs=mtok_sb[:, blk, :],
                                     start=(blk == 0), stop=False)
                for blk in range(NBm):
                    nc.tensor.matmul(out=cps[:], lhsT=gm[:, blk, :],
                                     rhs=mtm_sb[:, blk, :],
                                     start=False, stop=(blk == NBm - 1))
                compT = dpool.tile([P, L], F32, tag="compT")
                nc.vector.tensor_copy(out=compT[:], in_=cps[:])
                c2ps = ps_sm.tile([C, L], F32, space="PSUM", tag="sm")
                nc.tensor.matmul(out=c2ps[:], lhsT=w("w2"), rhs=compT[:],
                                 start=True, stop=False)
                nc.tensor.matmul(out=c2ps[:], lhsT=w("b2r"), rhs=ones16[:],
                                 start=False, stop=True)
                c2 = dpool.tile([C, L], F32, tag="c2")
                nc.vector.tensor_copy(out=c2[:], in_=c2ps[:])

                # ---------- LNf over the flattened [16*64] ----------
                junk = dpool.tile([C, L], F32, tag="junk")
                rs = dpool.tile([C, 1], F32, tag="rs")
                sqs = dpool.tile([C, 1], F32, tag="sqs")
                nc.scalar.activation(junk[:], c2[:], AF.Identity, accum_out=rs[:])
                nc.scalar.activation(junk[:], c2[:], AF.Square, accum_out=sqs[:])
                t1ps = ps_sm.tile([1, 1], F32, space="PSUM", tag="sm")
                t2ps = ps_sm.tile([1, 1], F32, space="PSUM", tag="sm")
                nc.tensor.matmul(out=t1ps[:], lhsT=rs[:], rhs=onescol[:],
                                 start=True, stop=True)
                nc.tensor.matmul(out=t2ps[:], lhsT=sqs[:], rhs=onescol[:],
                                 start=True, stop=True)
                mean = dpool.tile([1, 1], F32, tag="mean")
                msq = dpool.tile([1, 1], F32, tag="msq")
                nc.vector.tensor_scalar_mul(mean[:], t1ps[:], 1.0 / (L * C))
                nc.vector.tensor_scalar_mul(msq[:], t2ps[:], 1.0 / (L * C))
                var = dpool.tile([1, 1], F32, tag="var")
                nc.vector.tensor_tensor(out=var[:], in0=mean[:], in1=mean[:],
                                        op=ALU.mult)
                nc.vector.tensor_tensor(out=var[:], in0=msq[:], in1=var[:],
                                        op=ALU.subtract)
                rstd = dpool.tile([1, 1], F32, tag="rstd")
                nc.vector.tensor_scalar_add(rstd[:], var[:], EPS)
                nc.scalar.sqrt(rstd[:], rstd[:])
                nc.vector.reciprocal(rstd[:], rstd[:])
                nmr = dpool.tile([1, 1], F32, tag="nmr")
                nc.vector.tensor_scalar(out=nmr[:], in0=mean[:], scalar1=rstd[:],
                                        scalar2=-1.0, op0=ALU.mult, op1=ALU.mult)
                bc_r = dpool.tile([C, 1], F32, tag="bc_r")
                bc_n = dpool.tile([C, 1], F32, tag="bc_n")
                nc.gpsimd.partition_broadcast(bc_r[:], rstd[:])
                nc.gpsimd.partition_broadcast(bc_n[:], nmr[:])
                c2n = dpool.tile([C, L], F32, tag="c2n")
                nc.scalar.activation(c2n[:], c2[:], AF.Identity,
                                     bias=bc_n[:], scale=bc_r[:])
                if not cfg["lnf_triv"]:
                    nc.vector.tensor_tensor(out=c2n[:], in0=c2n[:],
                                            in1=w("lnfg"), op=ALU.mult)
                    nc.vector.tensor_tensor(out=c2n[:], in0=c2n[:],
                                            in1=w("lnfb"), op=ALU.add)

                # ---------- LNd per row + decode MLP (tiny) ----------
                cfps = ps_sm.tile([L, C], F32, space="PSUM", tag="sm")
                nc.tensor.transpose(out=cfps[:], in_=c2n[:], identity=ident[0:C, 0:C])
                cf = dpool.tile([L, C], F32, tag="cf")
                nc.vector.tensor_copy(out=cf[:], in_=cfps[:])
                st2 = dpool.tile([L, 6], F32, tag="st2")
                mv2 = dpool.tile([L, 2], F32, tag="mv2")
                nc.vector.bn_stats(st2[:], cf[:])
                nc.vector.bn_aggr(mv2[:], st2[:])
                rc2 = dpool.tile([L, 1], F32, tag="rc2")
                nm2 = dpool.tile([L, 1], F32, tag="nm2")
                nc.vector.tensor_scalar_add(rc2[:], mv2[:, 1:2], EPS)
                nc.scalar.sqrt(rc2[:], rc2[:])
                nc.vector.reciprocal(rc2[:], rc2[:])
                nc.vector.tensor_scalar(out=nm2[:], in0=mv2[:, 0:1], scalar1=rc2[:],
                                        scalar2=-1.0, op0=ALU.mult, op1=ALU.mult)
                t2n = dpool.tile([L, C], F32, tag="t2n")
                nc.scalar.activation(t2n[:], cf[:], AF.Identity,
                                     bias=nm2[:], scale=rc2[:])
                if not cfg["lnd_triv"]:
                    nc.vector.tensor_tensor(out=t2n[:], in0=t2n[:],
                                            in1=w("lndg"), op=ALU.mult)
                    nc.vector.tensor_tensor(out=t2n[:], in0=t2n[:],
                                            in1=w("lndb"), op=ALU.add)
                ttps = ps_sm.tile([C, L], F32, space="PSUM", tag="sm")
                nc.tensor.transpose(out=ttps[:], in_=t2n[:], identity=ident[0:L, 0:L])
                t2nT = dpool.tile([C, L], F32, tag="t2nT")
                nc.vector.tensor_copy(out=t2nT[:], in_=ttps[:])

                d1ps = ps_mm.tile([P, L], F32, space="PSUM", tag="pp")
                nc.tensor.matmul(out=d1ps[:], lhsT=w("dw1"), rhs=t2nT[:],
                                 start=True, stop=False)
                nc.tensor.matmul(out=d1ps[:], lhsT=w("db1r"), rhs=ones16[:],
                                 start=False, stop=True)
                d1T = dpool.tile([P, L], F32, tag="d1T")
                nc.scalar.activation(d1T[:], d1ps[:], AF.Gelu)
                decps = ps_out.tile([L, D], F32, space="PSUM", tag="ops")
                nc.tensor.matmul(out=decps[:], lhsT=d1T[:], rhs=w("dw2"),
                                 start=True, stop=False)
                nc.tensor.matmul(out=decps[:], lhsT=ones16[:], rhs=w("db2r"),
                                 start=False, stop=True)
                dec = dpool.tile([L, D], F32, tag="dec")
                nc.vector.tensor_copy(out=dec[:], in_=decps[:])
                dhl = dpool.tile([4 * L, D], BF16, tag="dhl")
                nc.vector.memset(dhl[:], 0.0)
                nc.vector.tensor_copy(out=dhl[0:L, :], in_=dec[:])
                dhi32 = dpool.tile([L, D], F32, tag="dhi32")
                nc.vector.tensor_copy(out=dhi32[:], in_=dhl[0:L, :])
                dlo = dpool.tile([L, D], F32, tag="dlo")
                nc.vector.tensor_tensor(out=dlo[:], in0=dec[:], in1=dhi32[:],
                                        op=ALU.subtract)
                nc.vector.tensor_copy(out=dhl[2 * L:3 * L, :], in_=dlo[:])

                if phase == 'dec':
                    nc.scalar.dma_start(out=out_d[0:32, :],
                                        in_=dhl[:].bitcast(F32))
                    for z in range(4):
                        nc.scalar.dma_start(
                            out=out_d[N_NODE + z * 1024:N_NODE + (z + 1) * 1024, :],
                            in_=zt[:])
                    return

                # ---------- output gather (one-hot matmul) + zeros ----------
                HB = B // 2
                for ht in range(2 * NBT):
                    ob = opool.tile([P, HB, D], F32, tag="ob")
                    for b in range(HB):
                        col = (ht * HB + b) * P
                        ops_ = ps_out.tile([P, D], F32, space="PSUM", tag="ops")
                        nc.tensor.matmul(out=ops_[:], lhsT=oh_sb[:, col:col + P],
                                         rhs=dhl[:], start=True, stop=True)
                        if b % 2 == 0:
                            nc.vector.tensor_copy(out=ob[:, b, :], in_=ops_[:])
                        else:
                            nc.scalar.copy(out=ob[:, b, :], in_=ops_[:])
                    nc.scalar.dma_start(
                        out=out_d[ht * TPB // 2:(ht + 1) * TPB // 2, :], in_=ob[:])
                for z in range(4):
                    nc.scalar.dma_start(
                        out=out_d[N_NODE + z * 1024:N_NODE + (z + 1) * 1024, :],
                        in_=zt[:])

            if loop_k > 0:
                with tc.For_i(0, loop_k, 1,
                              hint_engines=(mybir.EngineType.PE,
                                            mybir.EngineType.DVE,
                                            mybir.EngineType.Activation,
                                            mybir.EngineType.Pool,
                                            mybir.EngineType.SP)):
                    body()
            else:
                body()

    nc.compile()
    return nc


def kernel(**inputs) -> np.ndarray:
    cfg, in_maps = host_prep(**inputs)
    nc = build(cfg)
    res = bass_utils.run_bass_kernel_spmd(nc, in_maps, core_ids=list(range(T)))
    out = np.stack([res.results[c]["out"] for c in range(T)], axis=0)
    return out.astype(np.float32)


# revision 25
# speedup vs baseline: 1.7913x; 1.0138x over previous
"""Trainium2 Bass kernel for nn_Disentangler (gnn_message_passing).

Math (per timestamp t, fully data-parallel across 8 cores):
  xn   = LN(x[t, :8192], ln1_g, ln1_b)
  tee  = scatter_add(xn by indices[t]) into 32768 slots
  h    = gelu(tee @ w1 + b1) @ w2 + b2
  comp = LNf(chunk_sum(h))                       # 16 chunks of 2048 slots
  dec  = (gelu(LNd(comp_rows) @ dw1 + db1) @ dw2 + db2)   # only 16 distinct rows
  out[t, i] = dec[indices[t, i] >> 11]; out[t, 8192:] = 0

Key transforms vs the reference:
  * LN folds into the first matmul: a_i = r_i * ((x_i - m_i) @ (g*w1)) [+ b@w1]
  * slots hit by exactly one token need no scatter at all: their gelu(a_i)
    feeds the chunk-sum matmul directly in token order (M_tok).
  * only multi-hit slots (~12%) are accumulated, via multiplicity-ordered
    dma_gather rounds from a bf16 DRAM spill of `a` (M_mul matmul after gelu).
  * chunk-sum (incl. empty-slot correction via weighted fake rows) is a bf16
    matmul against host-built membership matrices.
  * decode MLP computed on 16 rows; the final gather is a one-hot matmul with
    a split-bf16 (hi+lo) trick for full fp32 precision.
"""

import math

import numpy as np
import ml_dtypes

import concourse.bacc as bacc
import concourse.mybir as mybir
import concourse.tile as tile
import concourse.bass_utils as bass_utils
from concourse.masks import make_identity

# problem constants
T, NUM_TOKENS, D = 8, 12288, 256
N_NODE, NUM_NODES = 8192, 32768
L, C, H = 16, 64, 128          # COMP_LEN, COMP_DIM, 2*COMP_DIM
CHUNK = NUM_NODES // L         # 2048
P = 128
B = 16                         # token blocks per bigtile (2048 tokens / bigtile)
NBT = N_NODE // (P * B)        # 4 bigtiles
NBLK = N_NODE // P             # 64 token blocks
ZROW = N_NODE                  # zero row in the a-spill table
EPS = 1e-5
F32 = mybir.dt.float32
BF16 = mybir.dt.bfloat16
I16 = mybir.dt.int16
AF = mybir.ActivationFunctionType
ALU = mybir.AluOpType
SINGLE_PACKET = False
COPY_ENG = 'mix'


def _pack16(v):
    """int index list (len % 16 == 0) -> [128, n/16] int16 (wrap 16, replicate)."""
    a = np.asarray(v, np.int16).reshape(-1, 16).T
    return np.ascontiguousarray(np.tile(a, (8, 1)))


def blob_layout(cfg):
    """fp32 weight blob layout: name -> (row0, nrows, col0, ncols)."""
    lay, col = {}, [0]

    def put(name, rows, cols):
        lay[name] = (0, rows, col[0], cols)
        col[0] += cols

    put("w1g0", P, H)
    put("w1g1", P, H)
    put("w2", H, C)
    put("dw1", C, H)
    put("dw2", H, D)
    put("b2r", 1, C)
    put("db1r", 1, H)
    put("db2r", 1, D)
    if not cfg["lnf_triv"]:
        put("lnfg", C, L)
        put("lnfb", C, L)
    if not cfg["lnd_triv"]:
        put("lndg", L, C)
        put("lndb", L, C)
    if cfg["has_bw1"]:
        put("bw1rep", P, H)
    if cfg["has_b1"]:
        put("b1rep", P, H)
    return lay, col[0]


def iblob_layout(cfg):
    """int16 blob: gather tables + bf16 membership matrices (bitcast)."""
    NBm, mks = cfg["NBm"], cfg["mks"]
    lay, col = {}, [0]

    def put(name, cols):
        lay[name] = (col[0], cols)
        col[0] += cols

    put("g0", cfg["Um_pad"] // 16)
    if cfg["K_g"] > 1:
        put("g1", mks[0] // 16)
    if cfg["K_g"] > 2:
        put("gt", sum(mks[1:]) // 16)
    put("mtok", NBLK * L)        # [128, 64*16] bf16
    put("mtm", NBm * L)          # [128, NBm*16] bf16
    put("w1gb0", H)              # [128, 128] bf16
    put("w1gb1", H)
    return lay, col[0]


def host_prep(x, indices, ln1_g, ln1_b, w1, b1, w2, b2,
              lnf_g, lnf_b, lnd_g, lnd_b, dw1, db1, dw2, db2):
    """Build per-core in_maps + global config."""
    f = np.float32
    x = np.asarray(x, f)
    ln1_g, ln1_b = np.asarray(ln1_g, f), np.asarray(ln1_b, f)
    w1, b1 = np.asarray(w1, f), np.asarray(b1, f)
    w2, b2 = np.asarray(w2, f), np.asarray(b2, f)
    lnf_g, lnf_b = np.asarray(lnf_g, f), np.asarray(lnf_b, f)
    lnd_g, lnd_b = np.asarray(lnd_g, f), np.asarray(lnd_b, f)
    dw1, db1 = np.asarray(dw1, f), np.asarray(db1, f)
    dw2, db2 = np.asarray(dw2, f), np.asarray(db2, f)

    per_t = []
    for t in range(T):
        idx = np.asarray(indices[t], np.int64)
        uniq, counts = np.unique(idx, return_counts=True)
        order = np.argsort(-counts, kind="stable")   # multi slots first
        sp = np.argsort(idx, kind="stable")
        starts = np.zeros(uniq.size + 1, np.int64)
        starts[1:] = np.cumsum(counts)
        per_t.append(dict(idx=idx, uniq=uniq, counts=counts, order=order,
                          sp=sp, starts=starts,
                          M=int((counts >= 2).sum()), K=int(counts.max())))

    K_g = max(d["K"] for d in per_t)
    M_max = max(d["M"] for d in per_t)
    Um_pad = P * math.ceil((M_max + L) / P)
    NBm = Um_pad // P
    mks = []
    for k in range(1, K_g):
        mk = max(int((d["counts"] > k).sum()) for d in per_t)
        mks.append(P * math.ceil(mk / P))

    cfg = dict(
        Um_pad=Um_pad, NBm=NBm, K_g=K_g, mks=mks,
        has_bw1=bool(np.any(ln1_b != 0)),
        has_b1=bool(np.any(b1 != 0)),
        lnf_triv=bool(np.all(lnf_g == 1) and np.all(lnf_b == 0)),
        lnd_triv=bool(np.all(lnd_g == 1) and np.all(lnd_b == 0)),
    )
    lay, wcols = blob_layout(cfg)
    ilay, icols = iblob_layout(cfg)
    cfg["wcols"], cfg["icols"] = wcols, icols

    W1g = ln1_g[:, None] * w1
    blob = np.zeros((P, wcols), np.float32)

    def put(name, val):
        r0, nr, c0, ncs = lay[name]
        blob[r0:r0 + nr, c0:c0 + ncs] = val

    put("w1g0", W1g[:P, :])
    put("w1g1", W1g[P:, :])
    put("w2", w2)
    put("dw1", dw1)
    put("dw2", dw2)
    put("b2r", (CHUNK * b2)[None, :])
    put("db1r", db1[None, :])
    put("db2r", db2[None, :])
    if not cfg["lnf_triv"]:
        put("lnfg", lnf_g.reshape(L, C).T)
        put("lnfb", lnf_b.reshape(L, C).T)
    if not cfg["lnd_triv"]:
        put("lndg", np.tile(lnd_g, (L, 1)))
        put("lndb", np.tile(lnd_b, (L, 1)))
    if cfg["has_bw1"]:
        put("bw1rep", np.tile((ln1_b @ w1)[None, :], (P, 1)))
    if cfg["has_b1"]:
        put("b1rep", np.tile(b1[None, :], (P, 1)))

    in_maps = []
    for t in range(T):
        d = per_t[t]
        idx, uniq, counts, order = d["idx"], d["uniq"], d["counts"], d["order"]
        sp, starts, M = d["sp"], d["starts"], d["M"]

        # gather tables (multi-hit slots only; desc-multiplicity prefix order)
        g0 = np.full(Um_pad, ZROW, np.int64)
        g0[:M] = sp[starts[order[:M]]]
        gks = []
        for k in range(1, K_g):
            gk = np.full(mks[k - 1], ZROW, np.int64)
            sel = counts[order] > k
            nsel = int(sel.sum())
            if nsel:
                gk[:nsel] = sp[starts[order[sel]] + k]
            gks.append(gk)

        # M_mul: multi-compact rows -> chunk, plus empty-correction fakes
        lu = (uniq >> 11).astype(np.int64)
        mtm = np.zeros((Um_pad, L), np.float32)
        mtm[np.arange(M), lu[order[:M]]] = 1.0
        cnt_chunk = np.bincount(lu, minlength=L).astype(np.float32)
        mtm[M + np.arange(L), np.arange(L)] = CHUNK - cnt_chunk
        mtm_dev = mtm.reshape(NBm, P, L).transpose(1, 0, 2).reshape(P, NBm * L)

        # M_tok: singleton-slot tokens -> chunk, in token order
        mtok = np.zeros((N_NODE, L), np.float32)
        sing = counts == 1
        spos = sp[starts[:-1][sing]]              # the single occurrence
        mtok[spos, lu[sing]] = 1.0
        # token = bt*2048 + p*16 + b  -> dev [p, bt*16+b, l]
        mtok_dev = (mtok.reshape(NBT, P, B, L).transpose(1, 0, 2, 3)
                    .reshape(P, NBLK * L))

        # output staging writes half-bigtiles: token = ht*1024 + p*8 + b
        l_arr = (idx >> 11).astype(np.int64)
        HB = B // 2
        lv = l_arr.reshape(2 * NBT, P, HB)
        oh = np.zeros((4 * L, 2 * NBT, HB, P), np.float32)
        ht_i, p_i, b_i = np.indices((2 * NBT, P, HB))
        oh[lv, ht_i, b_i, p_i] = 1.0
        oh[lv + 2 * L, ht_i, b_i, p_i] = 1.0
        oh_dev = oh.reshape(4 * L, N_NODE).astype(ml_dtypes.bfloat16)

        iblob = np.zeros((P, icols), np.int16)

        def iput(name, val):
            c0, ncs = ilay[name]
            iblob[:, c0:c0 + ncs] = val

        iput("g0", _pack16(g0))
        if K_g > 1:
            iput("g1", _pack16(gks[0]))
        if K_g > 2:
            iput("gt", np.concatenate([_pack16(g) for g in gks[1:]], axis=1))
        iput("mtok", np.tile(mtok_dev.astype(ml_dtypes.bfloat16)
                             .view(np.int16), (1, 1)))
        iput("mtm", mtm_dev.astype(ml_dtypes.bfloat16).view(np.int16))
        iput("w1gb0", W1g[:P, :].astype(ml_dtypes.bfloat16).view(np.int16))
        iput("w1gb1", W1g[P:, :].astype(ml_dtypes.bfloat16).view(np.int16))

        in_maps.append({
            "xt": np.ascontiguousarray(x[t, :N_NODE, :]),
            "oh": oh_dev,
            "iblob": np.ascontiguousarray(iblob),
            "wblob": blob,
        })
    return cfg, in_maps


def build(cfg, loop_k=0, phase='all'):
    """Build the Bass program. loop_k>0 wraps the body in a hardware loop
    (for timing); loop_k=0 emits a single-shot kernel."""
    Um_pad, NBm, K_g, mks = cfg["Um_pad"], cfg["NBm"], cfg["K_g"], cfg["mks"]
    lay, wcols = blob_layout(cfg)
    ilay, icols = iblob_layout(cfg)
    nc = bacc.Bacc("TRN2", num_devices=8, num_swdge_queues=2)

    xt = nc.dram_tensor("xt", [N_NODE, D], F32, kind="ExternalInput").ap()
    oh_d = nc.dram_tensor("oh", [4 * L, N_NODE], BF16, kind="ExternalInput").ap()
    ib_d = nc.dram_tensor("iblob", [P, icols], I16, kind="ExternalInput").ap()
    wb_d = nc.dram_tensor("wblob", [P, wcols], F32, kind="ExternalInput").ap()
    out_d = nc.dram_tensor("out", [NUM_TOKENS, D], F32, kind="ExternalOutput").ap()
    adram = nc.dram_tensor("adram", [N_NODE + 1, H], BF16, kind="Internal").ap()

    TPB = P * B  # tokens per bigtile

    with tile.TileContext(nc) as tc:
        with (
            tc.tile_pool(name="const", bufs=1) as cpool,
            tc.tile_pool(name="x", bufs=2) as xpool,
            tc.tile_pool(name="xc", bufs=2) as xcpool,
            tc.tile_pool(name="stats", bufs=2) as spool,
            tc.tile_pool(name="xT", bufs=2) as xtpool,
            tc.tile_pool(name="a", bufs=1) as apool,
            tc.tile_pool(name="ga", bufs=1) as gapool,
            tc.tile_pool(name="acc", bufs=1) as accpool,
            tc.tile_pool(name="stg", bufs=2) as stgpool,
            tc.tile_pool(name="dec", bufs=1) as dpool,
            tc.tile_pool(name="outp", bufs=2) as opool,
            tc.tile_pool(name="ps_tr", bufs=2, space="PSUM") as ps_tr,
            tc.tile_pool(name="ps_mm", bufs=2, space="PSUM") as ps_mm,
            tc.tile_pool(name="ps_cs", bufs=1, space="PSUM") as ps_cs,
            tc.tile_pool(name="ps_out", bufs=2, space="PSUM") as ps_out,
            tc.tile_pool(name="ps_sm", bufs=1, space="PSUM") as ps_sm,
        ):
            # ---------- constants ----------
            ident = cpool.tile([P, P], F32)
            make_identity(nc, ident[:])
            identb = cpool.tile([P, P], BF16)
            nc.vector.tensor_copy(out=identb[:], in_=ident[:])
            zt = cpool.tile([P, 2048], F32)
            nc.vector.memset(zt[:], 0.0)
            ones16 = cpool.tile([1, L], F32)
            nc.vector.memset(ones16[:], 1.0)
            onescol = cpool.tile([C, 1], F32)
            nc.vector.memset(onescol[:], 1.0)

            wb = cpool.tile([P, wcols], F32)
            nc.sync.dma_start(out=wb[:], in_=wb_d[:])

            def w(name):
                r0, nr, c0, ncs = lay[name]
                return wb[r0:r0 + nr, c0:c0 + ncs]

            ib = cpool.tile([P, icols], I16)
            nc.sync.dma_start(out=ib[:], in_=ib_d[:])

            def iw(name):
                c0, ncs = ilay[name]
                return ib[:, c0:c0 + ncs]

            mtok_sb = iw("mtok").bitcast(BF16).rearrange(
                "p (nb l) -> p nb l", l=L)
            mtm_sb = iw("mtm").bitcast(BF16).rearrange(
                "p (nb l) -> p nb l", l=L)
            oh_sb = cpool.tile([4 * L, N_NODE], BF16)
            nc.sync.dma_start(out=oh_sb[:], in_=oh_d[:])
            # zero row of the bf16 a-table
            nc.sync.dma_start(out=adram[ZROW:ZROW + 1, :],
                              in_=zt[0:1, 0:C].bitcast(BF16))

            LVL = {'null': 0, 'xload': 1, 'ln': 2, 'cen': 3, 'tr': 4,
                   'mm': 5, 'spill': 6, 'gather': 7, 'dec': 8, 'all': 9}
            lvl = LVL[phase]

            def body(_i=None):
                if lvl == 0:
                    nc.scalar.dma_start(out=out_d[0:1024, :], in_=zt[:])
                    return
                # ---------- encode: LN-folded matmul + bf16 spill ----------
                a_sb = apool.tile([P, NBLK, H], F32, tag="a")
                ga = gapool.tile([P, NBLK, H], BF16, tag="ga")
                for bt in range(NBT):
                    xb = xpool.tile([P, B, D], F32, tag="xb")
                    nc.sync.dma_start(
                        out=xb[:], in_=xt[bt * TPB:(bt + 1) * TPB, :])
                    if lvl <= 1:
                        continue
                    st = spool.tile([P, B, 6], F32, tag="st")
                    mv = spool.tile([P, B, 2], F32, tag="mv")
                    for b in range(B):
                        nc.vector.bn_stats(st[:, b, :], xb[:, b, :])
                    for b in range(B):
                        nc.vector.bn_aggr(mv[:, b, :], st[:, b, :])
                    rc = spool.tile([P, B], F32, tag="rc")
                    nm = spool.tile([P, B], F32, tag="nm")
                    nc.vector.tensor_scalar_add(rc[:], mv[:, :, 1], EPS)
                    nc.scalar.sqrt(rc[:], rc[:])
                    nc.vector.reciprocal(rc[:], rc[:])
                    nc.vector.tensor_scalar_mul(nm[:], mv[:, :, 0], -1.0)
                    if lvl <= 2:
                        continue
                    xc = xcpool.tile([P, B, D], BF16, tag="xc")
                    for b in range(B):
                        nc.vector.tensor_scalar(
                            out=xc[:, b, :], in0=xb[:, b, :],
                            scalar1=nm[:, b:b + 1], scalar2=rc[:, b:b + 1],
                            op0=ALU.add, op1=ALU.mult)
                    if lvl <= 3:
                        continue
                    trps, pps = [], []
                    for b in range(B):
                        trp = ps_tr.tile([P, D], BF16, space="PSUM", tag="trp")
                        trps.append(trp)
                        nc.tensor.transpose(
                            out=trp[:, 0:P], in_=xc[:, b, 0:P], identity=identb[:])
                        nc.tensor.transpose(
                            out=trp[:, P:D], in_=xc[:, b, P:D], identity=identb[:])
                    xTb = xtpool.tile([P, B, D], BF16, tag="xT")
                    for b in range(B):
                        dve = COPY_ENG == 'dve' or (COPY_ENG == 'mix' and b % 2 == 0)
                        if dve:
                            nc.vector.tensor_copy(out=xTb[:, b, :], in_=trps[b][:])
                        else:
                            nc.scalar.copy(out=xTb[:, b, :], in_=trps[b][:])
                    if lvl <= 4:
                        continue
                    for b in range(B):
                        pp = ps_mm.tile([P, H], F32, space="PSUM", tag="pp")
                        pps.append(pp)
                        nc.tensor.matmul(out=pp[:], lhsT=xTb[:, b, 0:P],
                                         rhs=iw("w1gb0").bitcast(BF16),
                                         start=True, stop=False)
                        nc.tensor.matmul(out=pp[:], lhsT=xTb[:, b, P:D],
                                         rhs=iw("w1gb1").bitcast(BF16),
                                         start=False, stop=True)
                        dve = COPY_ENG == 'dve' or (COPY_ENG == 'mix' and b % 2)
                        if dve:
                            nc.vector.tensor_copy(out=a_sb[:, bt * B + b, :],
                                                  in_=pp[:])
                        else:
                            nc.scalar.copy(out=a_sb[:, bt * B + b, :], in_=pp[:])
                    if cfg["has_bw1"]:
                        for b in range(B):
                            blk = bt * B + b
                            nc.vector.tensor_tensor(
                                out=a_sb[:, blk, :], in0=a_sb[:, blk, :],
                                in1=w("bw1rep"), op=ALU.add)
                    if lvl <= 5:
                        continue
                    # bf16 spill for the multi-slot gather (SWDGE cast)
                    nc.gpsimd.dma_start(
                        out=adram[bt * TPB:(bt + 1) * TPB, :],
                        in_=a_sb[:, bt * B:(bt + 1) * B, :])
                if lvl <= 5:
                    nc.scalar.dma_start(out=out_d[0:1024, :], in_=zt[:])
                    return
                # gelu(a [+ b1]) for the singleton path, cast to bf16
                if cfg["has_b1"]:
                    for blk in range(NBLK):
                        nc.vector.tensor_tensor(
                            out=a_sb[:, blk, :], in0=a_sb[:, blk, :],
                            in1=w("b1rep"), op=ALU.add)
                for blk0 in range(0, NBLK, 8):
                    nc.scalar.activation(
                        ga[:, blk0:blk0 + 8, :], a_sb[:, blk0:blk0 + 8, :],
                        AF.Gelu)

                if phase == 'spill':
                    for z in range(4):
                        nc.scalar.dma_start(
                            out=out_d[z * 1024:(z + 1) * 1024, :], in_=zt[:])
                    return

                # ---------- gather-accumulate multi-hit slots ----------
                acc = accpool.tile([P, NBm, H], BF16, tag="acc")
                nc.gpsimd.dma_gather(
                    acc[:], adram[:], iw("g0"), Um_pad, Um_pad, H,
                    single_packet=SINGLE_PACKET, queue_num=0)
                if K_g > 1:
                    nbk = mks[0] // P
                    stg = stgpool.tile([P, nbk, H], BF16, tag="stg")
                    nc.gpsimd.dma_gather(
                        stg[:], adram[:], iw("g1"), mks[0], mks[0], H,
                        single_packet=SINGLE_PACKET, queue_num=1)
                    nc.vector.tensor_tensor(
                        out=acc[:, 0:nbk, :], in0=acc[:, 0:nbk, :], in1=stg[:],
                        op=ALU.add)
                if K_g > 2:
                    ntail = sum(mks[1:])
                    stg2 = stgpool.tile([P, ntail // P, H], BF16, tag="stg2")
                    nc.gpsimd.dma_gather(
                        stg2[:], adram[:], iw("gt"), ntail, ntail, H,
                        single_packet=SINGLE_PACKET, queue_num=0)
                    off = 0
                    for k in range(2, K_g):
                        nbk = mks[k - 1] // P
                        nc.vector.tensor_tensor(
                            out=acc[:, 0:nbk, :], in0=acc[:, 0:nbk, :],
                            in1=stg2[:, off:off + nbk, :], op=ALU.add)
                        off += nbk
                if cfg["has_b1"]:
                    for blk in range(NBm):
                        nc.vector.tensor_tensor(
                            out=acc[:, blk, :], in0=acc[:, blk, :],
                            in1=w("b1rep"), op=ALU.add)
                gm = accpool.tile([P, NBm, H], BF16, tag="gm")
                nc.scalar.activation(gm[:], acc[:], AF.Gelu)

                if phase == 'gather':
                    nc.scalar.dma_start(out=out_d[0:NBm * 32, :],
                                        in_=gm[:].bitcast(F32))
                    for z in range(4):
                        nc.scalar.dma_start(
                            out=out_d[N_NODE + z * 1024:N_NODE + (z + 1) * 1024, :],
                            in_=zt[:])
                    return

                # ---------- chunk-sum matmul (tokens + multi) + w2 ----------
                cps = ps_cs.tile([P, L], F32, space="PSUM", tag="cps")
                for blk in range(NBLK):
                    nc.tensor.matmul(out=cps[:], lhsT=ga[:, blk, :],
                                     rhs=mtok_sb[:, blk, :],
                                     start=(blk == 0), stop=False)
                for blk in range(NBm):
                    nc.tensor.matmul(out=cps[:], lhsT=gm[:, blk, :],
                                     rhs=mtm_sb[:, blk, :],
                                     start=False, stop=(blk == NBm - 1))
                compT = dpool.tile([P, L], F32, tag="compT")
                nc.vector.tensor_copy(out=compT[:], in_=cps[:])
                c2ps = ps_sm.tile([C, L], F32, space="PSUM", tag="sm")
                nc.tensor.matmul(out=c2ps[:], lhsT=w("w2"), rhs=compT[:],
                                 start=True, stop=False)
                nc.tensor.matmul(out=c2ps[:], lhsT=w("b2r"), rhs=ones16[:],
                                 start=False, stop=True)
                c2 = dpool.tile([C, L], F32, tag="c2")
                nc.vector.tensor_copy(out=c2[:], in_=c2ps[:])

                # ---------- LNf over the flattened [16*64] ----------
                junk = dpool.tile([C, L], F32, tag="junk")
                rs = dpool.tile([C, 1], F32, tag="rs")
                sqs = dpool.tile([C, 1], F32, tag="sqs")
                nc.scalar.activation(junk[:], c2[:], AF.Identity, accum_out=rs[:])
                nc.scalar.activation(junk[:], c2[:], AF.Square, accum_out=sqs[:])
                t1ps = ps_sm.tile([1, 1], F32, space="PSUM", tag="sm")
                t2ps = ps_sm.tile([1, 1], F32, space="PSUM", tag="sm")
                nc.tensor.matmul(out=t1ps[:], lhsT=rs[:], rhs=onescol[:],
                                 start=True, stop=True)
                nc.tensor.matmul(out=t2ps[:], lhsT=sqs[:], rhs=onescol[:],
                                 start=True, stop=True)
                mean = dpool.tile([1, 1], F32, tag="mean")
                msq = dpool.tile([1, 1], F32, tag="msq")
                nc.vector.tensor_scalar_mul(mean[:], t1ps[:], 1.0 / (L * C))
                nc.vector.tensor_scalar_mul(msq[:], t2ps[:], 1.0 / (L * C))
                var = dpool.tile([1, 1], F32, tag="var")
                nc.vector.tensor_tensor(out=var[:], in0=mean[:], in1=mean[:],
                                        op=ALU.mult)
                nc.vector.tensor_tensor(out=var[:], in0=msq[:], in1=var[:],
                                        op=ALU.subtract)
                rstd = dpool.tile([1, 1], F32, tag="rstd")
                nc.vector.tensor_scalar_add(rstd[:], var[:], EPS)
                nc.scalar.sqrt(rstd[:], rstd[:])
                nc.vector.reciprocal(rstd[:], rstd[:])
                nmr = dpool.tile([1, 1], F32, tag="nmr")
                nc.vector.tensor_scalar(out=nmr[:], in0=mean[:], scalar1=rstd[:],
                                        scalar2=-1.0, op0=ALU.mult, op1=ALU.mult)
                bc_r = dpool.tile([C, 1], F32, tag="bc_r")
                bc_n = dpool.tile([C, 1], F32, tag="bc_n")
                nc.gpsimd.partition_broadcast(bc_r[:], rstd[:])
                nc.gpsimd.partition_broadcast(bc_n[:], nmr[:])
                c2n = dpool.tile([C, L], F32, tag="c2n")
                nc.scalar.activation(c2n[:], c2[:], AF.Identity,
                                     bias=bc_n[:], scale=bc_r[:])
                if not cfg["lnf_triv"]:
                    nc.vector.tensor_tensor(out=c2n[:], in0=c2n[:],
                                            in1=w("lnfg"), op=ALU.mult)
                    nc.vector.tensor_tensor(out=c2n[:], in0=c2n[:],
                                            in1=w("lnfb"), op=ALU.add)

                # ---------- LNd per row + decode MLP (tiny) ----------
                cfps = ps_sm.tile([L, C], F32, space="PSUM", tag="sm")
                nc.tensor.transpose(out=cfps[:], in_=c2n[:], identity=ident[0:C, 0:C])
                cf = dpool.tile([L, C], F32, tag="cf")
                nc.vector.tensor_copy(out=cf[:], in_=cfps[:])
                st2 = dpool.tile([L, 6], F32, tag="st2")
                mv2 = dpool.tile([L, 2], F32, tag="mv2")
                nc.vector.bn_stats(st2[:], cf[:])
                nc.vector.bn_aggr(mv2[:], st2[:])
                rc2 = dpool.tile([L, 1], F32, tag="rc2")
                nm2 = dpool.tile([L, 1], F32, tag="nm2")
                nc.vector.tensor_scalar_add(rc2[:], mv2[:, 1:2], EPS)
                nc.scalar.sqrt(rc2[:], rc2[:])
                nc.vector.reciprocal(rc2[:], rc2[:])
                nc.vector.tensor_scalar(out=nm2[:], in0=mv2[:, 0:1], scalar1=rc2[:],
                                        scalar2=-1.0, op0=ALU.mult, op1=ALU.mult)
                t2n = dpool.tile([L, C], F32, tag="t2n")
                nc.scalar.activation(t2n[:], cf[:], AF.Identity,
                                     bias=nm2[:], scale=rc2[:])
                if not cfg["lnd_triv"]:
                    nc.vector.tensor_tensor(out=t2n[:], in0=t2n[:],
                                            in1=w("lndg"), op=ALU.mult)
                    nc.vector.tensor_tensor(out=t2n[:], in0=t2n[:],
                                            in1=w("lndb"), op=ALU.add)
                ttps = ps_sm.tile([C, L], F32, space="PSUM", tag="sm")
                nc.tensor.transpose(out=ttps[:], in_=t2n[:], identity=ident[0:L, 0:L])
                t2nT = dpool.tile([C, L], F32, tag="t2nT")
                nc.vector.tensor_copy(out=t2nT[:], in_=ttps[:])

                d1ps = ps_mm.tile([P, L], F32, space="PSUM", tag="pp")
                nc.tensor.matmul(out=d1ps[:], lhsT=w("dw1"), rhs=t2nT[:],
                                 start=True, stop=False)
                nc.tensor.matmul(out=d1ps[:], lhsT=w("db1r"), rhs=ones16[:],
                                 start=False, stop=True)
                d1T = dpool.tile([P, L], F32, tag="d1T")
                nc.scalar.activation(d1T[:], d1ps[:], AF.Gelu)
                decps = ps_out.tile([L, D], F32, space="PSUM", tag="ops")
                nc.tensor.matmul(out=decps[:], lhsT=d1T[:], rhs=w("dw2"),
                                 start=True, stop=False)
                nc.tensor.matmul(out=decps[:], lhsT=ones16[:], rhs=w("db2r"),
                                 start=False, stop=True)
                dec = dpool.tile([L, D], F32, tag="dec")
                nc.vector.tensor_copy(out=dec[:], in_=decps[:])
                dhl = dpool.tile([4 * L, D], BF16, tag="dhl")
                nc.vector.memset(dhl[:], 0.0)
                nc.vector.tensor_copy(out=dhl[0:L, :], in_=dec[:])
                dhi32 = dpool.tile([L, D], F32, tag="dhi32")
                nc.vector.tensor_copy(out=dhi32[:], in_=dhl[0:L, :])
                dlo = dpool.tile([L, D], F32, tag="dlo")
                nc.vector.tensor_tensor(out=dlo[:], in0=dec[:], in1=dhi32[:],
                                        op=ALU.subtract)
                nc.vector.tensor_copy(out=dhl[2 * L:3 * L, :], in_=dlo[:])

                if phase == 'dec':
                    nc.scalar.dma_start(out=out_d[0:32, :],
                                        in_=dhl[:].bitcast(F32))
                    for z in range(4):
                        nc.scalar.dma_start(
                            out=out_d[N_NODE + z * 1024:N_NODE + (z + 1) * 1024, :],
                            in_=zt[:])
                    return

                # ---------- output gather (one-hot matmul) + zeros ----------
                HB = B // 2
                for ht in range(2 * NBT):
                    ob = opool.tile([P, HB, D], F32, tag="ob")
                    for b in range(HB):
                        col = (ht * HB + b) * P
                        ops_ = ps_out.tile([P, D], F32, space="PSUM", tag="ops")
                        nc.tensor.matmul(out=ops_[:], lhsT=oh_sb[:, col:col + P],
                                         rhs=dhl[:], start=True, stop=True)
                        if b % 2 == 0:
                            nc.vector.tensor_copy(out=ob[:, b, :], in_=ops_[:])
                        else:
                            nc.scalar.copy(out=ob[:, b, :], in_=ops_[:])
                    nc.scalar.dma_start(
                        out=out_d[ht * TPB // 2:(ht + 1) * TPB // 2, :], in_=ob[:])
                for z in range(4):
                    nc.scalar.dma_start(
                        out=out_d[N_NODE + z * 1024:N_NODE + (z + 1) * 1024, :],
                        in_=zt[:])

            if loop_k > 0:
                with tc.For_i(0, loop_k, 1,
                              hint_engines=(mybir.EngineType.PE,
                                            mybir.EngineType.DVE,
                                            mybir.EngineType.Activation,
                                            mybir.EngineType.Pool,
                                            mybir.EngineType.SP)):
                    body()
            else:
                body()

    nc.compile()
    return nc


def kernel(**inputs) -> np.ndarray:
    cfg, in_maps = host_prep(**inputs)
    nc = build(cfg)
    res = bass_utils.run_bass_kernel_spmd(nc, in_maps, core_ids=list(range(T)))
    out = np.stack([res.results[c]["out"] for c in range(T)], axis=0)
    return out.astype(np.float32)
